# revision 48
# baseline (speedup 1.0000x reference)
"""Distributed Trainium2 Bass kernel for nn_Attention_32246614458877.

Strategy (8 NeuronCores, (batch, kv-head) tensor parallel):
- Core r owns batch b=r//4 and kv-head g=r%4 (q heads 2g, 2g+1).
- All weights are pre-transposed + bf16-cast on the HOST into the exact
  lhsT/rhs DMA layouts the PE needs: zero on-device transposes. Every
  DRAM layout keeps sliced regions contiguous per partition so DMA
  descriptors stay 5-20KB (each queue serializes at ~80GB/s; small
  descriptors crater it).
- Startup DMAs are ~320-650KB chunks balanced across the gpsimd/scalar/
  sync queues in exactly matmul-consumption order, so the first matmul
  fires ~14us in and the PE chases the three parallel streams.
- (1+w) of the q/k rms-norm is folded into the weights on the host; the
  sum-of-squares runs as an all-ones-lhsT broadcasting matmul (result
  lands pre-broadcast across partitions) with 1/(1+w) riding the Square
  activation's per-partition scale.
- RoPE uses a single cos/sin half (the reference duplicates angles).
- Attention is flattened over (head, q-chunk, k-tile) and software-
  pipelined one tile ahead (scores(i+1) issue before PV(i)) so the Act
  exp latency never stalls the PE. Diagonal k-tiles compute only their
  causal q-slice (width 512-128*dt) for scores/exp/mask/PV/dn.
- The softmax denominator accumulates as a [128,512] all-ones-lhsT
  matmul (lands pre-broadcast, feeds reciprocal directly); full k-tile
  pairs share one dn matmul over a DVE-summed exp pair.
- attn^T re-shards to sequence via two 8-core AllToAlls (head 2g after
  its chunks, head 2g+1 at the end). Two full-size warm-up AllToAlls on
  the same buffers run early under projection compute: they absorb the
  cold-start CC cost (~70us on first execution) and resync the cores.
  Pack/unpack are few multi-dim DMAs via AP transposes; aoT keeps each
  unpack region contiguous (interval dep-tracking: no false A<->B deps).
  o_proj runs two passes of partial sums so pass 1 (head-2g columns)
  hides AllToAll #2. Output is bf16 (halves the writeback).
Compute dtype: bf16 operands with fp32 PSUM accumulation.
"""
import sys

sys.path.insert(0, "/opt/trn_rl_repo")
import numpy as np

B, S, D = 2, 2048, 2560
H, HKV, HD = 8, 4, 256
EPS = 1e-6
SCALING = 256 ** -0.5
NCORES = 8
SB = 2048           # sequence per batch (= per-core attention span)
DCH = D // 128      # 20 contraction chunks

_CACHE = {}


def _build():
    import concourse.bacc as bacc
    import concourse.mybir as mybir
    import concourse.tile as tile

    F32 = mybir.dt.float32
    BF16 = mybir.dt.bfloat16
    AF = mybir.ActivationFunctionType

    nc = bacc.Bacc("TRN2")

    xT_ext = nc.declare_dram_parameter("xT", [128, 4, DCH, 512], BF16,
                                       isOutput=False)
    cosT_ext = nc.declare_dram_parameter("cosT", [128, SB], BF16, isOutput=False)
    sinT_ext = nc.declare_dram_parameter("sinT", [128, SB], BF16, isOutput=False)
    qwT_ext = nc.declare_dram_parameter("qwT", [128, 4, DCH, 128], BF16, isOutput=False)
    kwT_ext = nc.declare_dram_parameter("kwT", [128, 2, DCH, 128], BF16, isOutput=False)
    vwT_ext = nc.declare_dram_parameter("vwT", [128, DCH, 256], BF16, isOutput=False)
    owT_ext = nc.declare_dram_parameter("owT", [128, 16, D], BF16, isOutput=False)
    # consts cols: [0:128] all-ones (ssq + dn broadcasts), [128:640] causal
    # mask slice, [640:642] q 1/(1+w) per-partition scale, [642:644] k ditto
    consts_ext = nc.declare_dram_parameter("consts", [128, 644], BF16,
                                           isOutput=False)
    out_ext = nc.declare_dram_parameter("out", [512, D], BF16, isOutput=True)

    GROUPS = [list(range(NCORES))]

    with tile.TileContext(nc) as tc:
        with (
            tc.tile_pool(name="const", bufs=1) as cpool,
            tc.tile_pool(name="persist", bufs=1) as ppool,
        ):
            # ---- persistent activations ----
            QT = ppool.tile([128, 4, SB], BF16)        # [hd128, 2h'+half, s]
            KT = ppool.tile([128, 2, SB], BF16)        # [hd128, half, s]
            Vf = ppool.tile([128, 16, 256], BF16)      # [kpos128, ktile, hd]

            # collective buffers (bf16 pairs packed as fp32)
            # 8-core AllToAll: target j owns q-slice [j*256,(j+1)*256) of
            # BOTH batches; A carries head 2g (lc 0,1), B head 2g+1.
            # Block layout [j, lc, p, x]: row j*256 + lc*128 + p.
            a2A_in = nc.dram_tensor("a2A_in", [8, 2, 128, 128], F32)[:]
            a2A_out = nc.dram_tensor("a2A_out", [2, 4, 2, 128, 128], F32)[:]
            a2B_in = nc.dram_tensor("a2B_in", [8, 2, 128, 128], F32)[:]
            a2B_out = nc.dram_tensor("a2B_out", [2, 4, 2, 128, 128], F32)[:]

            # ---- scoped pool for the projection phase ----
            proj_ctx = tc.tile_pool(name="projp", bufs=1)
            jpool = proj_ctx.__enter__()
            xT = jpool.tile([128, 4, DCH, 512], BF16, name="xT")
            cosT = jpool.tile([128, SB], BF16, name="cosT")
            sinT = jpool.tile([128, SB], BF16, name="sinT")
            qw_sb = jpool.tile([128, 4, DCH, 128], BF16, name="qw_sb")
            kw_sb = jpool.tile([128, 2, DCH, 128], BF16, name="kw_sb")
            vw_sb = jpool.tile([128, DCH, 256], BF16, name="vw_sb")
            consts = cpool.tile([128, 644], BF16, name="consts")

            # DMA order is per-queue program order; queues run in parallel
            # but each queue's transfers SERIALIZE at ~80GB/s, so split into
            # ~320-650KB chunks balanced across the three queues in exactly
            # the order the matmul stream consumes them.
            nc.gpsimd.dma_start(kw_sb[:, 0, 0:10, :], kwT_ext[:, 0, 0:10, :])
            nc.gpsimd.dma_start(kw_sb[:, 0, 10:20, :], kwT_ext[:, 0, 10:20, :])
            nc.gpsimd.dma_start(kw_sb[:, 1, 0:10, :], kwT_ext[:, 1, 0:10, :])
            nc.gpsimd.dma_start(kw_sb[:, 1, 10:20, :], kwT_ext[:, 1, 10:20, :])
            nc.gpsimd.dma_start(consts[:], consts_ext[:])
            nc.gpsimd.dma_start(qw_sb[:, 1, :, :], qwT_ext[:, 1, :, :])
            nc.gpsimd.dma_start(vw_sb[:, 10:20, :], vwT_ext[:, 10:20, :])
            nc.gpsimd.dma_start(xT[:, 1, 10:20, :], xT_ext[:, 1, 10:20, :])
            nc.gpsimd.dma_start(xT[:, 2, 10:20, :], xT_ext[:, 2, 10:20, :])
            nc.scalar.dma_start(xT[:, 0, 0:5, :], xT_ext[:, 0, 0:5, :])
            nc.scalar.dma_start(xT[:, 0, 5:10, :], xT_ext[:, 0, 5:10, :])
            nc.scalar.dma_start(qw_sb[:, 2, :, :], qwT_ext[:, 2, :, :])
            nc.scalar.dma_start(qw_sb[:, 3, :, :], qwT_ext[:, 3, :, :])
            nc.scalar.dma_start(xT[:, 1, 0:10, :], xT_ext[:, 1, 0:10, :])
            nc.scalar.dma_start(xT[:, 2, 0:10, :], xT_ext[:, 2, 0:10, :])
            nc.sync.dma_start(xT[:, 0, 10:15, :], xT_ext[:, 0, 10:15, :])
            nc.sync.dma_start(xT[:, 0, 15:20, :], xT_ext[:, 0, 15:20, :])
            nc.sync.dma_start(qw_sb[:, 0, 0:10, :], qwT_ext[:, 0, 0:10, :])
            nc.sync.dma_start(qw_sb[:, 0, 10:20, :], qwT_ext[:, 0, 10:20, :])
            nc.sync.dma_start(vw_sb[:, 0:10, :], vwT_ext[:, 0:10, :])
            nc.sync.dma_start(cosT[:], cosT_ext[:])
            nc.sync.dma_start(sinT[:], sinT_ext[:])
            nc.sync.dma_start(xT[:, 3, 0:10, :], xT_ext[:, 3, 0:10, :])
            nc.sync.dma_start(xT[:, 3, 10:20, :], xT_ext[:, 3, 10:20, :])
            # warm-up AllToAlls on the REAL buffers (garbage data), overlapped
            # under projection compute: they pay the cold CC/channel setup
            # and resync the cores' gpsimd queues long before the real
            # collectives. Only the gpsimd queue blocks, never the PE.
            nc.gpsimd.collective_compute(
                "AllToAll", mybir.AluOpType.bypass,
                replica_groups=GROUPS, ins=[a2A_in], outs=[a2A_out],
            )
            nc.gpsimd.collective_compute(
                "AllToAll", mybir.AluOpType.bypass,
                replica_groups=GROUPS, ins=[a2B_in], outs=[a2B_out],
            )

            ones128 = consts[:, 0:128]
            m512 = consts[:, 128:640]
            wscf = cpool.tile([128, 4], F32, name="wscf")
            nc.vector.tensor_copy(wscf[:], consts[:, 640:644])
            qsc = wscf[:, 0:2]
            ksc = wscf[:, 2:4]
            epsb = cpool.tile([128, 1], F32, name="epsb")
            nc.vector.memset(epsb[:], EPS)

            # ---- QKV projections + rms-norm + rope, n-chunk-major ----
            with (
                tc.tile_pool(name="phcs", bufs=2) as cspool,
                tc.tile_pool(name="phcps", bufs=2, space="PSUM") as cpsp,
                tc.tile_pool(name="phcps2", bufs=1, space="PSUM") as cpsp2,
                tc.tile_pool(name="phv", bufs=2, space="PSUM") as vpsp,
            ):
                units = [(w, h, n) for n in range(4)
                         for (w, h) in (("k", 0), ("q", 0), ("q", 1), ("v", 0))]
                pend_norm = [None]

                def emit_norm():
                    """Norm+rope of the previous qk unit, deferred until after
                    the next unit's first chain so the Square/add chain never
                    stalls the PE at the single ssqb matmul."""
                    if pend_norm[0] is None:
                        return
                    which, hh, n_, ps, wsc = pend_norm[0]
                    pend_norm[0] = None
                    ssqb = cpsp2.tile([128, 512], F32, tag="ssqb", bufs=1,
                                      name="ssqb")
                    sqs = []
                    for half in range(2):
                        sq = cspool.tile([128, 512], BF16, tag=f"sq{half}",
                                         bufs=2, name="sq")
                        nc.scalar.activation(sq[:], ps[half][:], AF.Square,
                                             scale=wsc[:, half:half + 1])
                        sqs.append(sq)
                    sqp = cspool.tile([128, 512], BF16, tag="sqp", bufs=2,
                                      name="sqp")
                    nc.vector.tensor_add(sqp[:], sqs[0][:], sqs[1][:])
                    nc.tensor.matmul(ssqb[:], ones128, sqp[:],
                                     start=True, stop=True)
                    sd = cspool.tile([128, 512], F32, tag="sd", name="sd")
                    nc.scalar.activation(sd[:], ssqb[:], AF.Sqrt,
                                         scale=1.0 / HD, bias=epsb[:, 0:1])
                    rsb = cspool.tile([128, 512], F32, tag="rsb", name="rsb")
                    nc.vector.reciprocal_approx_fast(rsb[:], sd[:])
                    bb = []
                    for half in range(2):
                        b = cspool.tile([128, 512], BF16, tag=f"b{half}",
                                        bufs=4, name="b")
                        nc.vector.tensor_mul(b[:], ps[half][:], rsb[:])
                        bb.append(b)
                    if which == "k":
                        d0 = KT[:, 0, n_ * 512:(n_ + 1) * 512]
                        d1 = KT[:, 1, n_ * 512:(n_ + 1) * 512]
                    else:
                        d0 = QT[:, hh * 2, n_ * 512:(n_ + 1) * 512]
                        d1 = QT[:, hh * 2 + 1, n_ * 512:(n_ + 1) * 512]
                    cs = cosT[:, n_ * 512:(n_ + 1) * 512]
                    sn = sinT[:, n_ * 512:(n_ + 1) * 512]
                    t0 = cspool.tile([128, 512], BF16, tag="t0", bufs=3,
                                     name="t0")
                    t1 = cspool.tile([128, 512], BF16, tag="t1", bufs=3,
                                     name="t1")
                    nc.vector.tensor_mul(t0[:], bb[0][:], cs)
                    nc.vector.tensor_mul(t1[:], bb[1][:], sn)
                    nc.vector.tensor_sub(d0, t0[:], t1[:])
                    t2 = cspool.tile([128, 512], BF16, tag="t0", bufs=3,
                                     name="t2")
                    t3 = cspool.tile([128, 512], BF16, tag="t1", bufs=3,
                                     name="t3")
                    nc.vector.tensor_mul(t2[:], bb[1][:], cs)
                    nc.vector.tensor_mul(t3[:], bb[0][:], sn)
                    nc.vector.tensor_add(d1, t2[:], t3[:])

                for which, hh, n_ in units:
                    if which == "v":
                        for j in range(4):
                            sc = 4 * n_ + j
                            vp = vpsp.tile([128, 256], F32, tag="vp")
                            for dc in range(DCH):
                                nc.tensor.matmul(
                                    vp[:],
                                    xT[:, n_, dc, j * 128:(j + 1) * 128],
                                    vw_sb[:, dc, :],
                                    start=(dc == 0), stop=(dc == DCH - 1),
                                )
                            if j == 0:
                                emit_norm()
                            nc.scalar.copy(Vf[:, sc, :], vp[:])
                        continue
                    wsb = kw_sb if which == "k" else qw_sb
                    wsc = ksc if which == "k" else qsc
                    ps = []
                    for half in range(2):
                        mi = hh * 2 + half
                        qkp = cpsp.tile([128, 512], F32, tag=f"qkp{half}")
                        for dc in range(DCH):
                            nc.tensor.matmul(
                                qkp[:],
                                wsb[:, mi, dc, :],
                                xT[:, n_, dc, :],
                                start=(dc == 0), stop=(dc == DCH - 1),
                            )
                        if half == 0:
                            emit_norm()
                        ps.append(qkp)
                    pend_norm[0] = (which, hh, n_, ps, wsc)
                emit_norm()

            proj_ctx.__exit__(None, None, None)

            # ---- o_w load + attention-phase tiles (overlaps attention) ----
            ow_ctx = tc.tile_pool(name="phow", bufs=1)
            owp = ow_ctx.__enter__()
            ow_sb = owp.tile([128, 16, D], BF16, name="ow_sb")
            attnT = owp.tile([128, 4, SB], BF16, name="attnT")  # [hd128, lc, q]
            # aoT[p, ab, bo, lc, gi, q] = attn^T[hd, batch bo, my q-slice]
            # from src core i = bo*4 + gi; head-chunk = 4*gi + 2*ab + lc.
            # ab/bo/lc outermost so each unpack DMA writes one contiguous
            # region (interval-based dep tracking: no false A<->B conflicts).
            aoT = owp.tile([128, 2, 2, 2, 4, 256], BF16, name="aoT")
            nc.sync.dma_start(ow_sb[:, 0:8, :], owT_ext[:, 0:8, :])
            nc.sync.dma_start(ow_sb[:, 8:16, :], owT_ext[:, 8:16, :])

            # ---- attention: flattened (head, q-chunk, k-tile), software-
            # pipelined one tile ahead; diagonal tiles causally sliced ----
            TILES = []
            for hh in range(2):
                for c in range(4):
                    ntiles = 4 * (c + 1)
                    for t in range(ntiles):
                        dt = t - (ntiles - 4)
                        qoff = 128 * dt if dt > 0 else 0
                        TILES.append((hh, c, t, qoff, dt >= 0,
                                      t == 0, t == ntiles - 1))

            def emit_pack_cc(head_sel):
                """Pack attnT halves for head_sel (0 -> lc 0,1 / 1 -> lc 2,3),
                run the AllToAll, unpack into aoT slots. Queue split keeps the
                in-order gpsimd queue from serializing A's unpacks against
                B's packs: A unpacks and B packs ride the idle sync queue."""
                lo = 2 * head_sel
                a_in = a2A_in if head_sel == 0 else a2B_in
                a_out = a2A_out if head_sel == 0 else a2B_out
                pack_qs = ((nc.gpsimd, nc.gpsimd) if head_sel == 0
                           else (nc.sync, nc.scalar))
                for lc in range(2):
                    src = attnT[:, lo + lc, :].bitcast(F32).rearrange(
                        "p (j x) -> p j x", j=8)
                    pack_qs[lc].dma_start(a_in[:, lc].transpose([1, 0, 2]),
                                          src)
                nc.gpsimd.collective_compute(
                    "AllToAll", mybir.AluOpType.bypass,
                    replica_groups=GROUPS,
                    ins=[a_in], outs=[a_out],
                )
                # bo=0 unpacks ride gpsimd (zero extra wait right after the
                # collective, and o_proj consumes bo=0 first); bo=1 on sync.
                for bo in range(2):
                    uq = nc.gpsimd if bo == 0 else nc.sync
                    for lc in range(2):
                        uq.dma_start(
                            aoT[:, head_sel, bo, lc, :, :].bitcast(F32),
                            a_out[bo, :, lc].transpose([1, 0, 2]))

            with (
                tc.tile_pool(name="phes", bufs=3) as espool,
                tc.tile_pool(name="pheps", bufs=2, space="PSUM") as epsp,
            ):
                cur = {}     # live psum tiles of the current chunk
                pend = None  # (tileinfo, P, ap0, ap1, dnp) awaiting PV+dn

                def emit_pv(p):
                    (hh, c, t, qoff, diag, first, last), P, ap0, ap1, dnp = p
                    st = first
                    nc.tensor.matmul(ap0[:, qoff:512], Vf[:, t, 0:128],
                                     P[:, qoff:512], start=st, stop=last)
                    nc.tensor.matmul(ap1[:, qoff:512], Vf[:, t, 128:256],
                                     P[:, qoff:512], start=st, stop=last)
                    # dn: diagonal tiles go per-tile (sliced); full tiles are
                    # paired - one all-ones matmul per DVE-summed pT pair
                    if diag:
                        nc.tensor.matmul(dnp[:, qoff:512], ones128,
                                         P[:, qoff:512], start=st, stop=last)
                    elif t % 2 == 0:
                        cur["dnPa"] = P
                    else:
                        pr = espool.tile([128, 512], BF16, tag="pr", bufs=2)
                        nc.vector.tensor_add(pr[:], cur["dnPa"][:], P[:])
                        nc.tensor.matmul(dnp[:], ones128, pr[:],
                                         start=(t == 1), stop=False)
                    if last:
                        rdb = espool.tile([128, 512], F32, tag="rdb", bufs=2)
                        nc.vector.reciprocal_approx_fast(rdb[:], dnp[:])
                        nc.vector.tensor_mul(
                            attnT[:, hh * 2, c * 512:(c + 1) * 512],
                            ap0[:], rdb[:])
                        nc.vector.tensor_mul(
                            attnT[:, hh * 2 + 1, c * 512:(c + 1) * 512],
                            ap1[:], rdb[:])
                        if hh == 0 and c == 3:
                            emit_pack_cc(0)

                for info in TILES:
                    hh, c, t, qoff, diag, first, last = info
                    if first:
                        cur = {
                            "ap0": epsp.tile([128, 512], F32, tag="ap0",
                                             bufs=2, name="ap0"),
                            "ap1": epsp.tile([128, 512], F32, tag="ap1",
                                             bufs=2, name="ap1"),
                            "dnp": epsp.tile([128, 512], F32, tag="dnp",
                                             bufs=1, name="dnp"),
                        }
                    sp = epsp.tile([128, 512], F32, tag="sp", bufs=3)
                    nc.tensor.matmul(sp[:, qoff:512],
                                     KT[:, 0, t * 128:(t + 1) * 128],
                                     QT[:, hh * 2, c * 512 + qoff:(c + 1) * 512],
                                     start=True, stop=False)
                    nc.tensor.matmul(sp[:, qoff:512],
                                     KT[:, 1, t * 128:(t + 1) * 128],
                                     QT[:, hh * 2 + 1, c * 512 + qoff:(c + 1) * 512],
                                     start=False, stop=True)
                    pT = espool.tile([128, 512], BF16, tag="pT", bufs=5)
                    nc.scalar.activation(pT[:, qoff:512], sp[:, qoff:512],
                                         AF.Exp, scale=SCALING)
                    P = pT
                    if diag:
                        pTm = espool.tile([128, 512], BF16, tag="pTm", bufs=3)
                        nc.vector.tensor_mul(pTm[:, qoff:512], pT[:, qoff:512],
                                             m512[:, 0:512 - qoff])
                        P = pTm
                    if pend is not None:
                        emit_pv(pend)
                    pend = (info, P, cur["ap0"], cur["ap1"], cur["dnp"])
                emit_pv(pend)

            # ---- AllToAll #2 (head 2g+1) ----
            emit_pack_cc(1)

            # ---- o_proj: two passes of partial sums so pass 1 (A-columns,
            # heads 2g) hides AllToAll #2 ----
            with (
                tc.tile_pool(name="pho", bufs=1) as opool,
                tc.tile_pool(name="phos", bufs=3) as ospool,
                tc.tile_pool(name="phops", bufs=3, space="PSUM") as opsp,
            ):
                part = opool.tile([128, 2, 2, 5, 512], F32)
                for bo in range(2):
                    for scl in range(2):
                        for do_ in range(5):
                            op = opsp.tile([128, 512], F32, tag="op", bufs=4)
                            i = 0
                            for gi in range(4):
                                for lc in range(2):
                                    nc.tensor.matmul(
                                        op[:],
                                        aoT[:, 0, bo, lc, gi,
                                            scl * 128:(scl + 1) * 128],
                                        ow_sb[:, 4 * gi + lc,
                                              do_ * 512:(do_ + 1) * 512],
                                        start=(i == 0), stop=(i == 7),
                                    )
                                    i += 1
                            nc.scalar.copy(part[:, bo, scl, do_, :], op[:])
                for bo in range(2):
                    for scl in range(2):
                        row0 = bo * 256 + scl * 128
                        for do_ in range(5):
                            op = opsp.tile([128, 512], F32, tag="op", bufs=4)
                            i = 0
                            for gi in range(4):
                                for lc in range(2):
                                    nc.tensor.matmul(
                                        op[:],
                                        aoT[:, 1, bo, lc, gi,
                                            scl * 128:(scl + 1) * 128],
                                        ow_sb[:, 4 * gi + 2 + lc,
                                              do_ * 512:(do_ + 1) * 512],
                                        start=(i == 0), stop=(i == 7),
                                    )
                                    i += 1
                            osb2 = ospool.tile([128, 512], BF16, tag="osb2")
                            last_tile = (bo == 1 and scl == 1 and do_ == 4)
                            if last_tile:
                                # split the final add+writeback so the tail
                                # pipeline drains ~1.5us sooner
                                for hf, hq in ((0, nc.sync), (1, nc.scalar)):
                                    sl = slice(hf * 256, (hf + 1) * 256)
                                    nc.vector.tensor_add(
                                        osb2[:, sl], op[:, sl],
                                        part[:, bo, scl, do_, sl])
                                    hq.dma_start(
                                        out_ext[row0:row0 + 128,
                                                do_ * 512 + hf * 256:
                                                do_ * 512 + (hf + 1) * 256],
                                        osb2[:, sl])
                            else:
                                nc.vector.tensor_add(osb2[:], op[:],
                                                     part[:, bo, scl, do_, :])
                                oq = nc.sync if do_ % 2 == 0 else nc.scalar
                                oq.dma_start(
                                    out_ext[row0:row0 + 128,
                                            do_ * 512:(do_ + 1) * 512],
                                    osb2[:])
            ow_ctx.__exit__(None, None, None)
    return nc


def _get_nc():
    if "nc" not in _CACHE:
        nc = _build()
        nc.finalize()
        _CACHE["nc"] = nc
    return _CACHE["nc"]


def _prepare_in_maps(x, cos, sin, q_w, k_w, v_w, o_w, qn_w, kn_w):
    import ml_dtypes
    BF = ml_dtypes.bfloat16
    x = np.asarray(x, np.float32)
    cos = np.asarray(cos, np.float32)
    sin = np.asarray(sin, np.float32)
    qn_w = np.asarray(qn_w, np.float32)
    kn_w = np.asarray(kn_w, np.float32)
    # fold the rms-norm (1+w) scaling into the projection weights
    q_w = np.asarray(q_w, np.float32) * np.tile(1.0 + qn_w, H)[:, None]
    k_w = np.asarray(k_w, np.float32) * np.tile(1.0 + kn_w, HKV)[:, None]
    v_w = np.asarray(v_w, np.float32)
    o_w = np.asarray(o_w, np.float32)

    xT, cosT, sinT = [], [], []
    for b in range(B):
        # [128, n(4), dc(20), 512]: each n-chunk contiguous per partition
        xb = np.ascontiguousarray(
            x[b].T.reshape(DCH, 128, 4, 512).transpose(1, 2, 0, 3)
        ).astype(BF)
        xT.append(np.ascontiguousarray(xb))
        # reference angles are duplicated across the two halves; keep one
        cosT.append(np.ascontiguousarray(cos[b, :, 0:128].T).astype(BF).copy())
        sinT.append(np.ascontiguousarray(sin[b, :, 0:128].T).astype(BF).copy())

    qwT, kwT, vwT = [], [], []
    for g in range(HKV):
        qg = q_w[g * 512:(g + 1) * 512]          # [512, 2560]
        qwT.append(np.ascontiguousarray(
            qg.reshape(4, 128, DCH, 128).transpose(3, 0, 2, 1)
        ).astype(BF).copy())
        kg = k_w[g * 256:(g + 1) * 256]
        kwT.append(np.ascontiguousarray(
            kg.reshape(2, 128, DCH, 128).transpose(3, 0, 2, 1)
        ).astype(BF).copy())
        vg = v_w[g * 256:(g + 1) * 256]          # [256, 2560]
        vwT.append(np.ascontiguousarray(
            vg.T.reshape(DCH, 128, 256).transpose(1, 0, 2)
        ).astype(BF).copy())

    owT = np.ascontiguousarray(
        o_w.T.reshape(16, 128, D).transpose(1, 0, 2)
    ).astype(BF).copy()

    # consts: [0:128] ones, [128:640] sliced causal mask (cols 0-127 tri,
    # 128+ ones), [640:642] q 1/(1+w) per-partition halves, [642:644] k ditto
    consts = np.empty((128, 644), np.float32)
    consts[:, 0:128] = 1.0
    p = np.arange(128).reshape(128, 1)
    j = np.arange(512).reshape(1, 512)
    consts[:, 128:640] = ((j >= 128) | (p <= j)).astype(np.float32)
    consts[:, 640:642] = (1.0 / (1.0 + qn_w)).reshape(2, 128).T
    consts[:, 642:644] = (1.0 / (1.0 + kn_w)).reshape(2, 128).T
    consts = consts.astype(BF)

    in_maps = []
    for r in range(NCORES):
        b, g = r // 4, r % 4
        in_maps.append({
            "xT": xT[b], "cosT": cosT[b], "sinT": sinT[b],
            "qwT": qwT[g], "kwT": kwT[g], "vwT": vwT[g], "owT": owT,
            "consts": consts,
        })
    return in_maps


def _run(trace=False):
    from concourse.bass_utils import run_bass_kernel_spmd
    nc = _get_nc()
    res = run_bass_kernel_spmd(nc, _CACHE["in_maps"], list(range(NCORES)),
                               trace=trace)
    outf = np.empty((B, S, D), np.float32)
    for r in range(NCORES):
        o = np.asarray(res.results[r]["out"], dtype=np.float32)
        for bo in range(B):
            outf[bo, r * 256:(r + 1) * 256] = o[bo * 256:(bo + 1) * 256]
    return outf, res


def kernel(x, cos, sin, mask, q_w, k_w, v_w, o_w, qn_w, kn_w):
    _CACHE["in_maps"] = _prepare_in_maps(x, cos, sin, q_w, k_w, v_w, o_w,
                                         qn_w, kn_w)
    out, _ = _run(trace=False)
    return out


def kernel_profiled(x, cos, sin, mask, q_w, k_w, v_w, o_w, qn_w, kn_w):
    _CACHE["in_maps"] = _prepare_in_maps(x, cos, sin, q_w, k_w, v_w, o_w,
                                         qn_w, kn_w)
    out, res = _run(trace=True)
    return out, res
